# revision 3
# baseline (speedup 1.0000x reference)
"""BEV feature extractor (bilinear gather) on 8 Trainium2 NeuronCores.

Hardcoded problem: bev_feature [4,180,180,512] f32, batch_centers [4,2500,2]
f32, num_point=5 -> out [4,500,2560] f32.

Sharding: data-parallel over batch, 2 cores per batch splitting the 500
output rows into halves of 250. Each core bilinearly samples 1250 points
from its batch's [180,180,512] map via SWDGE dma_gather (two 4KB-row
gathers per point: the (y0,x) and (y1,x) pixel pairs), applies the 4
bilinear weights on ACT/DVE, and writes its [250,5,512] output slice.
"""

import os

import numpy as np

H = W = 180
C = 512
B = 4
NPT = 2500
NUM_POINT = 5
SEC = 500          # points per channel-block
ROWS = H * W       # 32400 flat pixel rows
NCHUNK = 10        # device chunks of 128 point-slots
PADN = NCHUNK * 128
INV06 = np.float32(1.0) / np.float32(0.6)

_CACHE = {}
last_results = None  # BassKernelResults of the most recent run (for test.py)


def _build():
    import concourse.bacc as bacc
    import concourse.bass as bass
    import concourse.mybir as mybir
    import concourse.tile as tile
    from concourse.library_config import mlp

    f32 = mybir.dt.float32
    i32 = mybir.dt.int32
    i16 = mybir.dt.int16
    Alu = mybir.AluOpType

    nc = bacc.Bacc("TRN2", target_bir_lowering=False, debug=False)
    fmap = nc.dram_tensor("fmap", [ROWS, C], f32, kind="ExternalInput")
    c128 = nc.dram_tensor("c128", [128, 2 * NCHUNK], f32, kind="ExternalInput")
    c16 = nc.dram_tensor("c16", [128, 2 * PADN // 16], f32, kind="ExternalInput")
    out = nc.dram_tensor("out", [250, NUM_POINT, C], f32, kind="ExternalOutput")

    # overlapping pair-row view: row i covers flat pixel rows i and i+1
    fmap_view = bass.AP(fmap, 0, [[C, ROWS - 1], [1, 2 * C]])

    with tile.TileContext(nc) as tc:
        with (
            tc.tile_pool(name="pc", bufs=1) as pc,
            tc.tile_pool(name="pa", bufs=3) as pa,
            tc.tile_pool(name="pt", bufs=3) as pt,
            tc.tile_pool(name="po", bufs=3) as po,
        ):
            nc.gpsimd.load_library(mlp)

            ctr = pc.tile([128, 2 * NCHUNK], f32, tag="ctr")
            nc.sync.dma_start(ctr[:], c128[:])
            c16t = pc.tile([128, 2 * PADN // 16], f32, tag="c16t")
            nc.sync.dma_start(c16t[:], c16[:])

            def floor_pipe(src_x, src_y, n):
                """Returns (XS, YS, X0F, Y0F) [128, n] f32 tiles."""
                XS = pc.tile([128, n], f32, tag=f"XS{n}")
                nc.vector.tensor_scalar(XS[:], src_x, 54.0, float(INV06), Alu.add, Alu.mult)
                YS = pc.tile([128, n], f32, tag=f"YS{n}")
                nc.vector.tensor_scalar(YS[:], src_y, 54.0, float(INV06), Alu.add, Alu.mult)
                outs = []
                for S, nm in ((XS, "x"), (YS, "y")):
                    I0 = pc.tile([128, n], i32, tag=f"I0{nm}{n}")
                    nc.vector.tensor_copy(I0[:], S[:])
                    F0r = pc.tile([128, n], f32, tag=f"F0r{nm}{n}")
                    nc.vector.tensor_copy(F0r[:], I0[:])
                    # robust floor regardless of cast rounding: F0 = F0r - (F0r > S)
                    CR = pc.tile([128, n], f32, tag=f"CR{nm}{n}")
                    nc.vector.tensor_tensor(CR[:], F0r[:], S[:], Alu.is_gt)
                    F0 = pc.tile([128, n], f32, tag=f"F0{nm}{n}")
                    nc.vector.tensor_tensor(F0[:], F0r[:], CR[:], Alu.subtract)
                    outs.append(F0)
                return XS, YS, outs[0], outs[1]

            # ---- weight pipeline on [128, NCHUNK] ----
            xw = ctr[:][:, 0 : 2 * NCHUNK : 2]
            yw = ctr[:][:, 1 : 2 * NCHUNK : 2]
            XS, YS, X0F, Y0F = floor_pipe(xw, yw, NCHUNK)
            n = NCHUNK
            FX = pc.tile([128, n], f32, tag="FX")
            nc.vector.tensor_tensor(FX[:], XS[:], X0F[:], Alu.subtract)
            FY = pc.tile([128, n], f32, tag="FY")
            nc.vector.tensor_tensor(FY[:], YS[:], Y0F[:], Alu.subtract)
            X1F = pc.tile([128, n], f32, tag="X1F")
            nc.vector.tensor_scalar(X1F[:], X0F[:], 1.0, 179.0, Alu.add, Alu.min)
            Y1F = pc.tile([128, n], f32, tag="Y1F")
            nc.vector.tensor_scalar(Y1F[:], Y0F[:], 1.0, 179.0, Alu.add, Alu.min)
            MX = pc.tile([128, n], f32, tag="MX")
            nc.vector.tensor_scalar(MX[:], X0F[:], 178.0, None, Alu.is_le)
            MY = pc.tile([128, n], f32, tag="MY")
            nc.vector.tensor_scalar(MY[:], Y0F[:], 178.0, None, Alu.is_le)
            AX = pc.tile([128, n], f32, tag="AX")
            nc.vector.tensor_tensor(AX[:], X1F[:], XS[:], Alu.subtract)
            nc.vector.tensor_tensor(AX[:], AX[:], MX[:], Alu.mult)
            BX = pc.tile([128, n], f32, tag="BX")
            nc.vector.tensor_tensor(BX[:], FX[:], MX[:], Alu.mult)
            AY = pc.tile([128, n], f32, tag="AY")
            nc.vector.tensor_tensor(AY[:], Y1F[:], YS[:], Alu.subtract)
            nc.vector.tensor_tensor(AY[:], AY[:], MY[:], Alu.mult)
            BY = pc.tile([128, n], f32, tag="BY")
            nc.vector.tensor_tensor(BY[:], FY[:], MY[:], Alu.mult)
            WAA = pc.tile([128, n], f32, tag="WAA")
            nc.vector.tensor_tensor(WAA[:], AX[:], AY[:], Alu.mult)
            WAB = pc.tile([128, n], f32, tag="WAB")
            nc.vector.tensor_tensor(WAB[:], BX[:], AY[:], Alu.mult)
            WBA = pc.tile([128, n], f32, tag="WBA")
            nc.vector.tensor_tensor(WBA[:], AX[:], BY[:], Alu.mult)
            WBB = pc.tile([128, n], f32, tag="WBB")
            nc.vector.tensor_tensor(WBB[:], BX[:], BY[:], Alu.mult)

            # ---- index pipeline on [128, PADN//16] (16-partition replicated) ----
            m = PADN // 16
            x16 = c16t[:][:, 0 : 2 * m : 2]
            y16 = c16t[:][:, 1 : 2 * m : 2]
            _, _, X0F2, Y0F2 = floor_pipe(x16, y16, m)
            BXB = pc.tile([128, m], f32, tag="BXB")
            nc.vector.tensor_scalar(BXB[:], X0F2[:], 178.0, None, Alu.min)
            Y1F2 = pc.tile([128, m], f32, tag="Y1F2")
            nc.vector.tensor_scalar(Y1F2[:], Y0F2[:], 1.0, 179.0, Alu.add, Alu.min)
            IAf = pc.tile([128, m], f32, tag="IAf")
            nc.vector.tensor_scalar(IAf[:], Y0F2[:], 180.0, None, Alu.mult)
            nc.vector.tensor_tensor(IAf[:], IAf[:], BXB[:], Alu.add)
            IBf = pc.tile([128, m], f32, tag="IBf")
            nc.vector.tensor_scalar(IBf[:], Y1F2[:], 180.0, None, Alu.mult)
            nc.vector.tensor_tensor(IBf[:], IBf[:], BXB[:], Alu.add)
            IA16 = pc.tile([128, m], i16, tag="IA16")
            nc.vector.tensor_copy(IA16[:], IAf[:])
            IB16 = pc.tile([128, m], i16, tag="IB16")
            nc.vector.tensor_copy(IB16[:], IBf[:])

            # ---- per-chunk gather + weighted sum + store ----
            for k in range(NCHUNK):
                j, half = divmod(k, 2)
                cnt = 128 if half == 0 else 122
                A = pa.tile([128, 1, 2 * C], f32, tag="A")
                nc.gpsimd.dma_gather(
                    A[:], fmap_view, IA16[:, 8 * k : 8 * (k + 1)],
                    128, 128, 2 * C, elem_step=C,
                )
                Bt = pa.tile([128, 1, 2 * C], f32, tag="B")
                nc.gpsimd.dma_gather(
                    Bt[:], fmap_view, IB16[:, 8 * k : 8 * (k + 1)],
                    128, 128, 2 * C, elem_step=C,
                )
                t0 = pt.tile([128, C], f32, tag="t0")
                nc.scalar.mul(t0[:], A[:, 0, :C], WAA[:, k : k + 1])
                t1 = pt.tile([128, C], f32, tag="t1")
                nc.scalar.mul(t1[:], A[:, 0, C:], WAB[:, k : k + 1])
                s0 = pt.tile([128, C], f32, tag="s0")
                nc.vector.tensor_add(s0[:], t0[:], t1[:])
                t2 = pt.tile([128, C], f32, tag="t2")
                nc.scalar.mul(t2[:], Bt[:, 0, :C], WBA[:, k : k + 1])
                t3 = pt.tile([128, C], f32, tag="t3")
                nc.scalar.mul(t3[:], Bt[:, 0, C:], WBB[:, k : k + 1])
                s1 = pt.tile([128, C], f32, tag="s1")
                nc.vector.tensor_add(s1[:], t2[:], t3[:])
                o = po.tile([128, C], f32, tag="o")
                nc.vector.tensor_add(o[:], s0[:], s1[:])
                nc.sync.dma_start(
                    out[half * 128 : half * 128 + cnt, j, :], o[:cnt, :]
                )

    nc.compile()
    return nc


def _prep_core_inputs(fmap_b, cb, h):
    """fmap_b [ROWS, C] f32 view; cb [NPT, 2] f32; h in {0,1}."""
    pts = np.zeros((PADN, 2), np.float32)
    for k in range(NCHUNK):
        j, half = divmod(k, 2)
        cnt = 128 if half == 0 else 122
        p = np.arange(cnt)
        npt = j * SEC + h * 250 + half * 128 + p
        pts[k * 128 + p] = cb[npt]
    c128 = np.ascontiguousarray(
        pts.reshape(NCHUNK, 128, 2).transpose(1, 0, 2).reshape(128, 2 * NCHUNK)
    )
    c16 = np.ascontiguousarray(
        np.tile(pts.reshape(PADN // 16, 16, 2).transpose(1, 0, 2).reshape(16, -1), (8, 1))
    )
    return {"fmap": fmap_b, "c128": c128, "c16": c16}


def kernel(bev_feature, batch_centers, num_point=5):
    global last_results
    from concourse.bass_utils import run_bass_kernel_spmd

    assert int(num_point) == NUM_POINT
    bev = np.asarray(bev_feature, dtype=np.float32).reshape(B, ROWS, C)
    cen = np.asarray(batch_centers, dtype=np.float32)

    if "nc" not in _CACHE:
        _CACHE["nc"] = _build()
    nc = _CACHE["nc"]

    in_maps = []
    for c in range(8):
        b, h = divmod(c, 2)
        in_maps.append(_prep_core_inputs(bev[b], cen[b], h))

    trace = bool(os.environ.get("BEV_TRACE"))
    res = run_bass_kernel_spmd(nc, in_maps, list(range(8)), trace=trace)
    last_results = res

    full = np.empty((B, SEC, NUM_POINT * C), np.float32)
    for c in range(8):
        b, h = divmod(c, 2)
        full[b, h * 250 : (h + 1) * 250] = res.results[c]["out"].reshape(250, NUM_POINT * C)
    return full


# revision 19
# speedup vs baseline: 1.0139x; 1.0139x over previous
"""BEV feature extractor (bilinear gather) on 8 Trainium2 NeuronCores.

Hardcoded problem: bev_feature [4,180,180,512] f32, batch_centers [4,2500,2]
f32, num_point=5 -> out [4,500,2560] f32.

Sharding: data-parallel over batch, 2 cores per batch splitting the 500
output rows into halves of 250. Each core bilinearly samples 1250 points
from its batch's [180,180,512] map via SWDGE dma_gather (two 4KB-row
gathers per point: the (y0,x) and (y1,x) pixel pairs), applies the 4
bilinear weights on ACT/DVE, and writes its [250,5,512] output slice.
"""

import os

import numpy as np

H = W = 180
C = 512
B = 4
NPT = 2500
NUM_POINT = 5
SEC = 500          # points per channel-block
ROWS = H * W       # 32400 flat pixel rows
NCHUNK = 10        # device chunks of 128 point-slots
PADN = NCHUNK * 128
INV06 = np.float32(1.0) / np.float32(0.6)

_CACHE = {}
last_results = None  # BassKernelResults of the most recent run (for test.py)


def _build():
    import concourse.bacc as bacc
    import concourse.bass as bass
    import concourse.mybir as mybir
    import concourse.tile as tile
    from concourse.library_config import mlp

    f32 = mybir.dt.float32
    i32 = mybir.dt.int32
    i16 = mybir.dt.int16
    Alu = mybir.AluOpType

    m = PADN // 16  # 80 idx columns
    nc = bacc.Bacc("TRN2", target_bir_lowering=False, debug=False)
    fmap = nc.dram_tensor("fmap", [ROWS, C], f32, kind="ExternalInput")
    # cols 0:2*NCHUNK = per-partition point coords (weight layout),
    # cols 2*NCHUNK: = 16-partition-wrapped coords (idx layout, replicated x8)
    cent = nc.dram_tensor("cent", [128, 2 * NCHUNK + 2 * m], f32, kind="ExternalInput")
    out = nc.dram_tensor("out", [250, NUM_POINT, C], f32, kind="ExternalOutput")

    # overlapping pair-row view: row i covers flat pixel rows i and i+1
    fmap_view = bass.AP(fmap, 0, [[C, ROWS - 1], [1, 2 * C]])

    with tile.TileContext(nc) as tc:
        with (
            tc.tile_pool(name="pc", bufs=1) as pc,
            tc.tile_pool(name="pa", bufs=6) as pa,
            tc.tile_pool(name="pt", bufs=6) as pt,
            tc.tile_pool(name="po", bufs=4) as po,
        ):
            nc.gpsimd.load_library(mlp)

            ctr = pc.tile([128, 2 * NCHUNK + 2 * m], f32, tag="ctr")
            nc.sync.dma_start(ctr[:], cent[:])

            def floor_of(S, nm, n):
                """f32 floor of integer-range positive S, robust to the DVE
                converter's round-to-nearest."""
                I0 = pc.tile([128, n], i32, tag=f"I0{nm}{n}")
                nc.vector.tensor_copy(I0[:], S)
                F0r = pc.tile([128, n], f32, tag=f"F0r{nm}{n}")
                nc.vector.tensor_copy(F0r[:], I0[:])
                CR = pc.tile([128, n], f32, tag=f"CR{nm}{n}")
                nc.vector.tensor_tensor(CR[:], F0r[:], S, Alu.is_gt)
                F0 = pc.tile([128, n], f32, tag=f"F0{nm}{n}")
                nc.vector.tensor_tensor(F0[:], F0r[:], CR[:], Alu.subtract)
                return F0

            # ---- index pipeline on [128, m] (16-partition replicated) ----
            # processed in column halves so the first gathers launch while
            # the second half's indices are still being computed
            IDX = pc.tile([128, 2 * m], i16, tag="IDX")
            idx_v = IDX[:].rearrange("p (k two h) -> p k two h", two=2, h=8)
            Gs = []
            for hh in range(2):
                mh = m // 2
                co = 2 * NCHUNK + hh * 2 * mh
                x16 = ctr[:][:, co + 0 : co + 2 * mh : 2]
                y16 = ctr[:][:, co + 1 : co + 2 * mh : 2]
                XS2 = pc.tile([128, mh], f32, tag=f"XS2{hh}")
                nc.vector.tensor_scalar(XS2[:], x16, 54.0, float(INV06), Alu.add, Alu.mult)
                YS2 = pc.tile([128, mh], f32, tag=f"YS2{hh}")
                nc.vector.tensor_scalar(YS2[:], y16, 54.0, float(INV06), Alu.add, Alu.mult)
                X0F2 = floor_of(XS2[:], f"x{hh}", mh)
                Y0F2 = floor_of(YS2[:], f"y{hh}", mh)
                BXB = pc.tile([128, mh], f32, tag=f"BXB{hh}")
                nc.vector.tensor_scalar(BXB[:], X0F2[:], 178.0, None, Alu.min)
                Y1F2 = pc.tile([128, mh], f32, tag=f"Y1F2{hh}")
                nc.vector.tensor_scalar(Y1F2[:], Y0F2[:], 1.0, 179.0, Alu.add, Alu.min)
                IAf = pc.tile([128, mh], f32, tag=f"IAf{hh}")
                nc.vector.scalar_tensor_tensor(IAf[:], Y0F2[:], 180.0, BXB[:], Alu.mult, Alu.add)
                IBf = pc.tile([128, mh], f32, tag=f"IBf{hh}")
                nc.vector.scalar_tensor_tensor(IBf[:], Y1F2[:], 180.0, BXB[:], Alu.mult, Alu.add)
                # interleaved idx cols 16k..16k+8 = A-pair idxs, +8..+16 = B-pair
                kv = idx_v[:, hh * NCHUNK // 2 : (hh + 1) * NCHUNK // 2]
                nc.vector.tensor_copy(kv[:, :, 0, :], IAf[:].rearrange("p (k h) -> p k h", h=8))
                nc.vector.tensor_copy(kv[:, :, 1, :], IBf[:].rearrange("p (k h) -> p k h", h=8))
                for k in range(hh * NCHUNK // 2, (hh + 1) * NCHUNK // 2):
                    G = pa.tile([128, 2, 2 * C], f32, tag="G")
                    nc.gpsimd.dma_gather(
                        G[:], fmap_view, IDX[:, 16 * k : 16 * (k + 1)],
                        256, 256, 2 * C, elem_step=C,
                    )
                    Gs.append(G)

            # ---- weight pipeline on [128, NCHUNK] ----
            xw = ctr[:][:, 0 : 2 * NCHUNK : 2]
            yw = ctr[:][:, 1 : 2 * NCHUNK : 2]
            n = NCHUNK
            XS = pc.tile([128, n], f32, tag="XS")
            nc.vector.tensor_scalar(XS[:], xw, 54.0, float(INV06), Alu.add, Alu.mult)
            nc.vector.tensor_scalar(XS[:], XS[:], 179.0, None, Alu.min)
            YS = pc.tile([128, n], f32, tag="YS")
            nc.vector.tensor_scalar(YS[:], yw, 54.0, float(INV06), Alu.add, Alu.mult)
            nc.vector.tensor_scalar(YS[:], YS[:], 179.0, None, Alu.min)
            X0F = floor_of(XS[:], "xw", n)
            Y0F = floor_of(YS[:], "yw", n)
            FX = pc.tile([128, n], f32, tag="FX")
            nc.vector.tensor_tensor(FX[:], XS[:], X0F[:], Alu.subtract)
            FY = pc.tile([128, n], f32, tag="FY")
            nc.vector.tensor_tensor(FY[:], YS[:], Y0F[:], Alu.subtract)
            X1F = pc.tile([128, n], f32, tag="X1F")
            nc.vector.tensor_scalar(X1F[:], X0F[:], 1.0, 179.0, Alu.add, Alu.min)
            Y1F = pc.tile([128, n], f32, tag="Y1F")
            nc.vector.tensor_scalar(Y1F[:], Y0F[:], 1.0, 179.0, Alu.add, Alu.min)
            AX = pc.tile([128, n], f32, tag="AX")
            nc.vector.tensor_tensor(AX[:], X1F[:], XS[:], Alu.subtract)
            AY = pc.tile([128, n], f32, tag="AY")
            nc.vector.tensor_tensor(AY[:], Y1F[:], YS[:], Alu.subtract)
            WAA = pc.tile([128, n], f32, tag="WAA")
            nc.vector.tensor_tensor(WAA[:], AX[:], AY[:], Alu.mult)
            WAB = pc.tile([128, n], f32, tag="WAB")
            nc.vector.tensor_tensor(WAB[:], FX[:], AY[:], Alu.mult)
            WBA = pc.tile([128, n], f32, tag="WBA")
            nc.vector.tensor_tensor(WBA[:], AX[:], FY[:], Alu.mult)
            WBB = pc.tile([128, n], f32, tag="WBB")
            nc.vector.tensor_tensor(WBB[:], FX[:], FY[:], Alu.mult)

            # ---- per-chunk weighted sum + store ----
            for k in range(NCHUNK):
                j, half = divmod(k, 2)
                cnt = 128 if half == 0 else 122
                G = Gs[k]
                # 3 muls on ACT, FMA + 2 adds on DVE
                t0 = pt.tile([128, C], f32, tag="t0")
                nc.scalar.mul(t0[:], G[:, 0, :C], WAA[:, k : k + 1])
                t1 = pt.tile([128, C], f32, tag="t1")
                nc.scalar.mul(t1[:], G[:, 0, C:], WAB[:, k : k + 1])
                t2 = pt.tile([128, C], f32, tag="t2")
                nc.scalar.mul(t2[:], G[:, 1, :C], WBA[:, k : k + 1])
                s0 = pt.tile([128, C], f32, tag="s0")
                nc.vector.scalar_tensor_tensor(
                    s0[:], G[:, 1, C:], WBB[:, k : k + 1], t0[:], Alu.mult, Alu.add
                )
                s1 = pt.tile([128, C], f32, tag="s1")
                nc.vector.tensor_add(s1[:], s0[:], t1[:])
                o = po.tile([128, C], f32, tag="o")
                nc.vector.tensor_add(o[:], s1[:], t2[:])
                nc.sync.dma_start(
                    out[half * 128 : half * 128 + cnt, j, :], o[:cnt, :]
                )

    nc.compile()
    return nc


def _prep_core_inputs(fmap_b, cb, h):
    """fmap_b [ROWS, C] f32 view; cb [NPT, 2] f32; h in {0,1}."""
    pts = np.zeros((PADN, 2), np.float32)
    for k in range(NCHUNK):
        j, half = divmod(k, 2)
        cnt = 128 if half == 0 else 122
        p = np.arange(cnt)
        npt = j * SEC + h * 250 + half * 128 + p
        pts[k * 128 + p] = cb[npt]
    c128 = pts.reshape(NCHUNK, 128, 2).transpose(1, 0, 2).reshape(128, 2 * NCHUNK)
    c16 = np.tile(pts.reshape(PADN // 16, 16, 2).transpose(1, 0, 2).reshape(16, -1), (8, 1))
    cent = np.ascontiguousarray(np.concatenate([c128, c16], axis=1))
    return {"fmap": fmap_b, "cent": cent}


def kernel(bev_feature, batch_centers, num_point=5):
    global last_results
    from concourse.bass_utils import run_bass_kernel_spmd

    assert int(num_point) == NUM_POINT
    bev = np.asarray(bev_feature, dtype=np.float32).reshape(B, ROWS, C)
    cen = np.asarray(batch_centers, dtype=np.float32)

    if "nc" not in _CACHE:
        _CACHE["nc"] = _build()
    nc = _CACHE["nc"]

    in_maps = []
    for c in range(8):
        b, h = divmod(c, 2)
        in_maps.append(_prep_core_inputs(bev[b], cen[b], h))

    trace = bool(os.environ.get("BEV_TRACE"))
    res = run_bass_kernel_spmd(nc, in_maps, list(range(8)), trace=trace)
    last_results = res

    full = np.empty((B, SEC, NUM_POINT * C), np.float32)
    for c in range(8):
        b, h = divmod(c, 2)
        full[b, h * 250 : (h + 1) * 250] = res.results[c]["out"].reshape(250, NUM_POINT * C)
    return full


# revision 23
# speedup vs baseline: 1.0278x; 1.0137x over previous
"""BEV feature extractor (bilinear gather) on 8 Trainium2 NeuronCores.

Hardcoded problem: bev_feature [4,180,180,512] f32, batch_centers [4,2500,2]
f32, num_point=5 -> out [4,500,2560] f32.

Sharding: data-parallel over batch, 2 cores per batch splitting the 500
output rows into halves of 250. Each core bilinearly samples 1250 points
from its batch's [180,180,512] map via SWDGE dma_gather (two 4KB-row
gathers per point: the (y0,x) and (y1,x) pixel pairs), applies the 4
bilinear weights on ACT/DVE, and writes its [250,5,512] output slice.
"""

import os

import numpy as np

H = W = 180
C = 512
B = 4
NPT = 2500
NUM_POINT = 5
SEC = 500          # points per channel-block
ROWS = H * W       # 32400 flat pixel rows
NCHUNK = 10        # device chunks of 128 point-slots
PADN = NCHUNK * 128
INV06 = np.float32(1.0) / np.float32(0.6)

_CACHE = {}
last_results = None  # BassKernelResults of the most recent run (for test.py)


def _build():
    import concourse.bacc as bacc
    import concourse.bass as bass
    import concourse.mybir as mybir
    import concourse.tile as tile
    from concourse.library_config import mlp

    f32 = mybir.dt.float32
    i32 = mybir.dt.int32
    i16 = mybir.dt.int16
    Alu = mybir.AluOpType

    m = PADN // 16  # 80 idx columns
    nc = bacc.Bacc("TRN2", target_bir_lowering=False, debug=False)
    fmap = nc.dram_tensor("fmap", [ROWS, C], f32, kind="ExternalInput")
    # cols 0:2*NCHUNK = per-partition point coords (weight layout),
    # cols 2*NCHUNK: = 16-partition-wrapped coords (idx layout, replicated x8)
    cent = nc.dram_tensor("cent", [128, 2 * NCHUNK + 2 * m], f32, kind="ExternalInput")
    out = nc.dram_tensor("out", [250, NUM_POINT, C], f32, kind="ExternalOutput")

    # overlapping pair-row view: row i covers flat pixel rows i and i+1
    fmap_view = bass.AP(fmap, 0, [[C, ROWS - 1], [1, 2 * C]])

    with tile.TileContext(nc) as tc:
        with (
            tc.tile_pool(name="pc", bufs=1) as pc,
            tc.tile_pool(name="pa", bufs=6) as pa,
            tc.tile_pool(name="pt", bufs=6) as pt,
            tc.tile_pool(name="po", bufs=4) as po,
        ):
            nc.gpsimd.load_library(mlp)

            ctr = pc.tile([128, 2 * NCHUNK + 2 * m], f32, tag="ctr")
            nc.sync.dma_start(ctr[:], cent[:])

            def floor_of(S, nm, n):
                """f32 floor of integer-range positive S, robust to the DVE
                converter's round-to-nearest."""
                I0 = pc.tile([128, n], i32, tag=f"I0{nm}{n}")
                nc.vector.tensor_copy(I0[:], S)
                F0r = pc.tile([128, n], f32, tag=f"F0r{nm}{n}")
                nc.vector.tensor_copy(F0r[:], I0[:])
                CR = pc.tile([128, n], f32, tag=f"CR{nm}{n}")
                nc.vector.tensor_tensor(CR[:], F0r[:], S, Alu.is_gt)
                F0 = pc.tile([128, n], f32, tag=f"F0{nm}{n}")
                nc.vector.tensor_tensor(F0[:], F0r[:], CR[:], Alu.subtract)
                return F0

            # ---- index pipeline on [128, m] (16-partition replicated) ----
            # processed in column halves so the first gathers launch while
            # the second half's indices are still being computed.
            # centers arrive as grid coords (host does the /0.075/8 with
            # correctly-rounded f32 division, matching the CPU reference).
            IDX = pc.tile([128, 2 * m], i16, tag="IDX")
            idx_v = IDX[:].rearrange("p (k two h) -> p k two h", two=2, h=8)
            Gs = []
            for hh in range(2):
                mh = m // 2
                co = 2 * NCHUNK + hh * 2 * mh
                x16 = ctr[:][:, co + 0 : co + 2 * mh : 2]
                y16 = ctr[:][:, co + 1 : co + 2 * mh : 2]
                X0F2 = floor_of(x16, f"x{hh}", mh)
                Y0F2 = floor_of(y16, f"y{hh}", mh)
                BXB = pc.tile([128, mh], f32, tag=f"BXB{hh}")
                nc.vector.tensor_scalar(BXB[:], X0F2[:], 178.0, None, Alu.min)
                Y1F2 = pc.tile([128, mh], f32, tag=f"Y1F2{hh}")
                nc.vector.tensor_scalar(Y1F2[:], Y0F2[:], 1.0, 179.0, Alu.add, Alu.min)
                IAf = pc.tile([128, mh], f32, tag=f"IAf{hh}")
                nc.vector.scalar_tensor_tensor(IAf[:], Y0F2[:], 180.0, BXB[:], Alu.mult, Alu.add)
                IBf = pc.tile([128, mh], f32, tag=f"IBf{hh}")
                nc.vector.scalar_tensor_tensor(IBf[:], Y1F2[:], 180.0, BXB[:], Alu.mult, Alu.add)
                # interleaved idx cols 16k..16k+8 = A-pair idxs, +8..+16 = B-pair
                kv = idx_v[:, hh * NCHUNK // 2 : (hh + 1) * NCHUNK // 2]
                nc.vector.tensor_copy(kv[:, :, 0, :], IAf[:].rearrange("p (k h) -> p k h", h=8))
                nc.vector.tensor_copy(kv[:, :, 1, :], IBf[:].rearrange("p (k h) -> p k h", h=8))
                for k in range(hh * NCHUNK // 2, (hh + 1) * NCHUNK // 2):
                    G = pa.tile([128, 2, 2 * C], f32, tag="G")
                    nc.gpsimd.dma_gather(
                        G[:], fmap_view, IDX[:, 16 * k : 16 * (k + 1)],
                        256, 256, 2 * C, elem_step=C,
                    )
                    Gs.append(G)

            # ---- weight pipeline on [128, NCHUNK] ----
            xw = ctr[:][:, 0 : 2 * NCHUNK : 2]
            yw = ctr[:][:, 1 : 2 * NCHUNK : 2]
            n = NCHUNK
            XS = pc.tile([128, n], f32, tag="XS")
            nc.vector.tensor_scalar(XS[:], xw, 179.0, None, Alu.min)
            YS = pc.tile([128, n], f32, tag="YS")
            nc.vector.tensor_scalar(YS[:], yw, 179.0, None, Alu.min)
            X0F = floor_of(XS[:], "xw", n)
            Y0F = floor_of(YS[:], "yw", n)
            FX = pc.tile([128, n], f32, tag="FX")
            nc.vector.tensor_tensor(FX[:], XS[:], X0F[:], Alu.subtract)
            FY = pc.tile([128, n], f32, tag="FY")
            nc.vector.tensor_tensor(FY[:], YS[:], Y0F[:], Alu.subtract)
            X1F = pc.tile([128, n], f32, tag="X1F")
            nc.vector.tensor_scalar(X1F[:], X0F[:], 1.0, 179.0, Alu.add, Alu.min)
            Y1F = pc.tile([128, n], f32, tag="Y1F")
            nc.vector.tensor_scalar(Y1F[:], Y0F[:], 1.0, 179.0, Alu.add, Alu.min)
            AX = pc.tile([128, n], f32, tag="AX")
            nc.vector.tensor_tensor(AX[:], X1F[:], XS[:], Alu.subtract)
            AY = pc.tile([128, n], f32, tag="AY")
            nc.vector.tensor_tensor(AY[:], Y1F[:], YS[:], Alu.subtract)
            WAA = pc.tile([128, n], f32, tag="WAA")
            nc.vector.tensor_tensor(WAA[:], AX[:], AY[:], Alu.mult)
            WAB = pc.tile([128, n], f32, tag="WAB")
            nc.vector.tensor_tensor(WAB[:], FX[:], AY[:], Alu.mult)
            WBA = pc.tile([128, n], f32, tag="WBA")
            nc.vector.tensor_tensor(WBA[:], AX[:], FY[:], Alu.mult)
            WBB = pc.tile([128, n], f32, tag="WBB")
            nc.vector.tensor_tensor(WBB[:], FX[:], FY[:], Alu.mult)

            # ---- per-chunk weighted sum + store ----
            for k in range(NCHUNK):
                j, half = divmod(k, 2)
                cnt = 128 if half == 0 else 122
                G = Gs[k]
                # 3 muls on ACT, FMA + 2 adds on DVE
                t0 = pt.tile([128, C], f32, tag="t0")
                nc.scalar.mul(t0[:], G[:, 0, :C], WAA[:, k : k + 1])
                t1 = pt.tile([128, C], f32, tag="t1")
                nc.scalar.mul(t1[:], G[:, 0, C:], WAB[:, k : k + 1])
                t2 = pt.tile([128, C], f32, tag="t2")
                nc.scalar.mul(t2[:], G[:, 1, :C], WBA[:, k : k + 1])
                s0 = pt.tile([128, C], f32, tag="s0")
                nc.vector.scalar_tensor_tensor(
                    s0[:], G[:, 1, C:], WBB[:, k : k + 1], t0[:], Alu.mult, Alu.add
                )
                s1 = pt.tile([128, C], f32, tag="s1")
                nc.vector.tensor_add(s1[:], s0[:], t1[:])
                o = po.tile([128, C], f32, tag="o")
                nc.vector.tensor_add(o[:], s1[:], t2[:])
                nc.sync.dma_start(
                    out[half * 128 : half * 128 + cnt, j, :], o[:cnt, :]
                )

    nc.compile()
    return nc


def _prep_core_inputs(fmap_b, cb, h):
    """fmap_b [ROWS, C] f32 view; cb [NPT, 2] f32 GRID coords; h in {0,1}."""
    pts = np.full((PADN, 2), np.float32(90.0))
    for k in range(NCHUNK):
        j, half = divmod(k, 2)
        cnt = 128 if half == 0 else 122
        p = np.arange(cnt)
        npt = j * SEC + h * 250 + half * 128 + p
        pts[k * 128 + p] = cb[npt]
    c128 = pts.reshape(NCHUNK, 128, 2).transpose(1, 0, 2).reshape(128, 2 * NCHUNK)
    c16 = np.tile(pts.reshape(PADN // 16, 16, 2).transpose(1, 0, 2).reshape(16, -1), (8, 1))
    cent = np.ascontiguousarray(np.concatenate([c128, c16], axis=1))
    return {"fmap": fmap_b, "cent": cent}


def kernel(bev_feature, batch_centers, num_point=5):
    global last_results
    from concourse.bass_utils import run_bass_kernel_spmd

    assert int(num_point) == NUM_POINT
    bev = np.asarray(bev_feature, dtype=np.float32).reshape(B, ROWS, C)
    cen = np.asarray(batch_centers, dtype=np.float32)
    # grid coords, computed exactly like the f32 reference: (c+54)/0.075/8
    cen = (cen - np.float32(-54.0)) / np.float32(0.075) / np.float32(8.0)

    if "nc" not in _CACHE:
        _CACHE["nc"] = _build()
    nc = _CACHE["nc"]

    in_maps = []
    for c in range(8):
        b, h = divmod(c, 2)
        in_maps.append(_prep_core_inputs(bev[b], cen[b], h))

    trace = bool(os.environ.get("BEV_TRACE"))
    res = run_bass_kernel_spmd(nc, in_maps, list(range(8)), trace=trace)
    last_results = res

    full = np.empty((B, SEC, NUM_POINT * C), np.float32)
    for c in range(8):
        b, h = divmod(c, 2)
        full[b, h * 250 : (h + 1) * 250] = res.results[c]["out"].reshape(250, NUM_POINT * C)
    return full
